# revision 20
# baseline (speedup 1.0000x reference)
"""Trainium2 Bass kernel: DarkChannelLoss.

Computes -mean(dark_channel(x)) for x [32,3,512,512] f32, where
dark_channel = reflect-pad(7) -> min over channels -> 15x15 sliding-window
min (windows clipped at bottom/right, i.e. +inf padded by 14).

Sharding: pure data parallel over batch, 4 images per NeuronCore x 8 cores.
Each core computes column-partial sums of its dark-channel map (f32, via
TensorE ones-matmul accumulation in PSUM); the host combines them into the
final scalar mean.

Per-core pipeline (shapes hardcoded), images processed in pairs (bi=2):
  load:   8 up-front SWDGE cast DMAs (f32->bf16), one per (pair, row-tile):
          1.5 MB each, all resident in SBUF (48 KB/partition). Issued before
          any compute so GpSimd descriptor-gen never contends with DVE
          2-port instructions mid-kernel.
  pass 1: per row-tile: channel-min, reflect pads along W (ACT reversed
          copies), sliding-min cascade along W (windows 2,4,8,15), ops
          carry both images of the pair via a middle AP dim.
  transpose: TensorE 128x128 blocks (identity matmul) into per-W-tile PSUM
          banks [128, (img,rowtile), 128]; ScalarE evacuation per W-tile.
  pass 2: per W-tile: sliding-min cascade along H; dc tiles are summed by
          TensorE matmul with a ones-vector into one PSUM f32 accumulator
          (start/stop chain) -- zero DVE cost. The narrow last W-tile
          (14 cols) of all 4 images is packed into one tile at partition
          offsets {0,32,64,96} via ScalarE cross-quad copies, swept once.
  out:    [1, 512] f32 partial column sums; host reduces.

Emission order interleaves ACT evacuations between the two pairs' DVE
pass-1 blocks so no engine stream ever stalls on a later-emitted op.
"""

import numpy as np

try:
    import concourse.bass as bass
except ImportError:  # pragma: no cover
    import sys

    sys.path.insert(0, "/opt/trn_rl_repo")
    import concourse.bass as bass

import concourse.mybir as mybir
import concourse.bacc as bacc
from concourse.tile import TileContext
from concourse.bass_utils import run_bass_kernel_spmd

F32 = mybir.dt.float32
BF16 = mybir.dt.bfloat16
INF = float("inf")
MIN = mybir.AluOpType.min

B, C, H, W = 32, 3, 512, 512
WIN = 15
PAD = WIN // 2          # 7
HP = H + 2 * PAD        # 526 padded rows
WP = W + 2 * PAD        # 526 padded cols
N_CORES = 8
N_IMG = B // N_CORES    # 4 images per core
NT = H // 128           # 4 row tiles of source rows
PT = (WP + 127) // 128  # 5 W tiles
FREE = 544              # rm tile free width
MF = 544                # m/cascade tile free width (col = src_w + 8)
PL = 272                # even/odd half-plane length (270 used: rows 0..539)
DEN = B * HP * WP


def build_program(n_img=N_IMG, bi=2):
    assert n_img % bi == 0
    nb = n_img // bi
    nc = bacc.Bacc("TRN2", target_bir_lowering=False, debug=False)
    x = nc.dram_tensor("x", [n_img, C, H, W], F32, kind="ExternalInput")
    out = nc.dram_tensor("out", [1, 512], F32, kind="ExternalOutput")

    n2w = WP + WIN - 2  # 539
    n4w = n2w - 2
    n8w = n4w - 4

    with TileContext(nc) as tc:
        from contextlib import ExitStack

        with ExitStack() as ctx:
            constp = ctx.enter_context(tc.tile_pool(name="const", bufs=1))
            chp = ctx.enter_context(tc.tile_pool(name="ch", bufs=2))
            tmpp = ctx.enter_context(tc.tile_pool(name="tmp", bufs=2))
            mp = ctx.enter_context(tc.tile_pool(name="m", bufs=2))
            cascp = ctx.enter_context(tc.tile_pool(name="casc", bufs=4))
            rmp = ctx.enter_context(tc.tile_pool(name="rm", bufs=3))
            tbp = ctx.enter_context(tc.tile_pool(name="tb", bufs=1))
            dcp = ctx.enter_context(tc.tile_pool(name="dc", bufs=3))
            accp = ctx.enter_context(tc.tile_pool(name="acc", bufs=1))
            psp = ctx.enter_context(tc.tile_pool(name="ps", bufs=1, space="PSUM"))

            # ---- all loads up-front: one SWDGE cast DMA per (pair, row-tile),
            # emitted FIRST so SWDGE descriptor-gen starts at program start.
            # tag per row-tile, bufs=2 -> each (pair, row-tile) gets its own
            # resident buffer (48 KB/partition total)
            ch = [
                [
                    chp.tile([128, bi, C, W], BF16, tag=f"ch{t}", name=f"ch_{b}_{t}")
                    for t in range(NT)
                ]
                for b in range(nb)
            ]
            # Pace descriptor generation: before generating chunk k's
            # descriptors, wait for chunk k-2 to land. With >2 chunks queued
            # the 16 SDMA engines round-robin across them at packet
            # granularity, smearing every completion to the latest chunk's
            # finish time and starving pass 1.
            pace = constp.tile([128, 2 * NT], BF16, tag="pace")
            chunks = [(b, t) for b in range(nb) for t in range(NT)]
            for k, (b, t) in enumerate(chunks):
                if k >= 2:
                    pb, pt = chunks[k - 2]
                    nc.gpsimd.tensor_scalar_add(
                        pace[0:1, k : k + 1], ch[pb][pt][0:1, 0, 0, 0:1], 0.0
                    )
                nc.gpsimd.dma_start(
                    ch[b][t][:, :, :, :],
                    x[bi * b : bi * (b + 1), :, 128 * t : 128 * (t + 1), :]
                    .rearrange("b c p w -> p b c w"),
                )

            ident = constp.tile([128, 128], BF16, tag="ident")
            idt = constp.tile([128, 128], mybir.dt.int16, tag="idt")
            nc.gpsimd.iota(idt[:, :], pattern=[[1, 128]], base=0, channel_multiplier=-1)
            nc.vector.tensor_single_scalar(
                ident[:, :], idt[:, :], 0, mybir.AluOpType.is_equal
            )
            ones = constp.tile([128, 1], BF16, tag="ones")
            nc.vector.memset(ones[:, :], 1.0)

            # Transposed buffers hold H rows split into even/odd half-planes
            # (plane length PL): row r=2j   -> plane 0 index j
            #                    row r=2j+1 -> plane 1 index j
            # This keeps every pass-2 operand unit-stride for DVE 2x mode.
            # packed tile for the narrow last W-tile of all n_img images:
            # image i sits at partitions 32i..32i+14; other lanes stay 0.0
            tbP4 = accp.tile([128, 1, 2, PL], BF16, tag="tbP4")
            nc.vector.memset(tbP4[:, :, :, :], 0.0)

            # persistent transposed buffers, one per pair; INF tail for the
            # clipped bottom windows set once here (rows 526..539 = planes
            # j in [263, 270))
            tbm = [
                tbp.tile([128, PT - 1, bi, 2, PL], BF16, tag=f"tbm{b}", name=f"tbm{b}")
                for b in range(nb)
            ]
            tb4 = [
                tbp.tile([128, bi, 2, PL], BF16, tag=f"tb4{b}", name=f"tb4{b}")
                for b in range(nb)
            ]
            for b in range(nb):
                nc.vector.memset(tbm[b][:, :, :, :, 263:PL], INF)
                nc.vector.memset(tb4[b][0:14, :, :, 263:PL], INF)

            pst = [
                [
                    psp.tile(
                        [128, bi, NT, 128], BF16, tag=f"pst{p}", name=f"pst_{b}_{p}"
                    )
                    for p in range(PT)
                ]
                for b in range(nb)
            ]

            def pass1(b):
                for t in range(NT):
                    tmp = tmpp.tile([128, bi, W], BF16, tag="tmp", name=f"tmp_{b}_{t}")
                    nc.vector.tensor_tensor(
                        tmp[:, :, :], ch[b][t][:, :, 0], ch[b][t][:, :, 1], MIN
                    )
                    m = mp.tile([128, bi, MF], BF16, tag="m", name=f"m_{b}_{t}")
                    nc.vector.memset(m[:, :, 8 + W : MF], INF)
                    nc.vector.tensor_tensor(
                        m[:, :, 8 : 8 + W], tmp[:, :, :], ch[b][t][:, :, 2], MIN
                    )
                    # reflect pads: padded 0..6 <- cols 15..9; 519..525 <- 518..512
                    # done on DVE (min(x,x)=x) so the ACT stream has no
                    # pass-1 dependencies and evacuations never stall it
                    nc.vector.tensor_tensor(
                        m[:, :, 1:8], m[:, :, 15:8:-1], m[:, :, 15:8:-1], MIN
                    )
                    nc.vector.tensor_tensor(
                        m[:, :, 520:527], m[:, :, 518:511:-1], m[:, :, 518:511:-1], MIN
                    )

                    w2 = cascp.tile([128, bi, MF], BF16, tag="casc", name=f"w2_{b}_{t}")
                    w4 = cascp.tile([128, bi, MF], BF16, tag="casc", name=f"w4_{b}_{t}")
                    w8 = cascp.tile([128, bi, MF], BF16, tag="casc", name=f"w8_{b}_{t}")
                    nc.vector.tensor_tensor(
                        w2[:, :, 0:n2w], m[:, :, 1 : n2w + 1], m[:, :, 2 : n2w + 2], MIN
                    )
                    nc.vector.tensor_tensor(
                        w4[:, :, 0:n4w], w2[:, :, 0:n4w], w2[:, :, 2 : n4w + 2], MIN
                    )
                    nc.vector.tensor_tensor(
                        w8[:, :, 0:n8w], w4[:, :, 0:n8w], w4[:, :, 4 : n8w + 4], MIN
                    )
                    rm = rmp.tile([128, bi, FREE], BF16, tag="rm", name=f"rm_{b}_{t}")
                    nc.vector.tensor_tensor(
                        rm[:, :, 0:512], w8[:, :, 0:512], w8[:, :, PAD : 512 + PAD], MIN
                    )
                    nc.vector.tensor_tensor(
                        rm[:, :, 512:WP], w8[:, :, 512:WP], w8[:, :, 512 + PAD : WP + PAD], MIN
                    )
                    # transposes for this row-tile: 2 images x (4 full 128-col
                    # W-blocks + one narrow 14-col block)
                    for ii in range(bi):
                        for p in range(PT - 1):
                            nc.tensor.transpose(
                                pst[b][p][:, ii, t, :],
                                rm[:, ii, 128 * p : 128 * (p + 1)],
                                ident[:, :],
                            )
                        nc.tensor.transpose(
                            pst[b][PT - 1][0:14, ii, t, :],
                            rm[:, ii, 512:WP],
                            ident[:, :],
                        )

            def evac(b):
                # PSUM rows r = 128t + w + PAD:
                #   w odd  (w=2u+1) -> even plane, j = 64t + u + 4
                #   w even (w=2u)   -> odd plane,  j = 64t + u + 3
                for p in range(PT):
                    if p < PT - 1:
                        src, npart = pst[b][p], 128
                        tbv = tbm[b][:, p]
                    else:
                        src, npart = pst[b][p][0:14], 14
                        tbv = tb4[b][0:14, :]
                    nc.scalar.copy(
                        tbv[:, :, 0, 4:260].rearrange("a b (t u) -> a b t u", t=NT),
                        src[0:npart, :, :, 1:128:2],
                    )
                    nc.scalar.copy(
                        tbv[:, :, 1, 3:259].rearrange("a b (t u) -> a b t u", t=NT),
                        src[0:npart, :, :, 0:128:2],
                    )
                    # row reflection per plane:
                    #   top: rows 0..6 <- 14..8; bottom: rows 519..525 <- 517..511
                    nc.scalar.copy(tbv[:, :, 0, 0:4], tbv[:, :, 0, 7:3:-1])
                    nc.scalar.copy(tbv[:, :, 1, 0:3], tbv[:, :, 1, 6:3:-1])
                    nc.scalar.copy(tbv[:, :, 0, 260:263], tbv[:, :, 0, 258:255:-1])
                    nc.scalar.copy(tbv[:, :, 1, 259:263], tbv[:, :, 1, 258:254:-1])
                    if p == PT - 1:
                        for ii in range(bi):
                            i = bi * b + ii
                            nc.scalar.copy(
                                tbP4[32 * i : 32 * i + 14, 0, :, :],
                                tb4[b][0:14, ii, :, :],
                            )

            mm_state = {"first": True}

            def sum_matmuls(dc_ap, chunks, last=False):
                # accumulate column sums of dc_ap chunks (slices of the last
                # dim) into sacc via ones-matmul; chunk free size must be <=512
                outer = 1
                for d in dc_ap.shape[1:-1]:
                    outer *= d
                for ci, (lo, hi) in enumerate(chunks):
                    n = outer * (hi - lo)
                    rhs = dc_ap[..., lo:hi]
                    nc.tensor.matmul(
                        sacc[0:1, 0:n],
                        ones[:, :],
                        rhs,
                        start=mm_state["first"],
                        stop=last and ci == len(chunks) - 1,
                        skip_group_check=True,
                    )
                    mm_state["first"] = False

            def eo_cascade(E, O, p2, s2, s4, s7, dce, dco):
                # sliding-min-15 over rows via even/odd planes ([128, x, PL] APs):
                #   p2[j]=min(rows 2j,2j+1); s7[j]=min(p2[j..j+6])
                #   dc_e[j]=min(s7[j],   E[j+7])  (rows 2j..2j+14)
                #   dc_o[j]=min(O[j],    s7[j+1]) (rows 2j+1..2j+15)
                nc.vector.tensor_tensor(p2[:, :, 0:270], E[:, :, 0:270], O[:, :, 0:270], MIN)
                nc.vector.tensor_tensor(s2[:, :, 0:269], p2[:, :, 0:269], p2[:, :, 1:270], MIN)
                nc.vector.tensor_tensor(s4[:, :, 0:267], s2[:, :, 0:267], s2[:, :, 2:269], MIN)
                nc.vector.tensor_tensor(s7[:, :, 0:264], s4[:, :, 0:264], s4[:, :, 3:267], MIN)
                nc.vector.tensor_tensor(dce[:, :, 0:263], s7[:, :, 0:263], E[:, :, 7:270], MIN)
                nc.vector.tensor_tensor(dco[:, :, 0:263], O[:, :, 0:263], s7[:, :, 1:264], MIN)

            def pass2(b):
                for p in range(PT - 1):
                    tbv = tbm[b][:, p]
                    p2 = cascp.tile([128, bi, PL], BF16, tag="eoc", name=f"p2_{b}_{p}")
                    s2 = cascp.tile([128, bi, PL], BF16, tag="eoc", name=f"s2_{b}_{p}")
                    s4 = cascp.tile([128, bi, PL], BF16, tag="eoc", name=f"s4_{b}_{p}")
                    s7 = cascp.tile([128, bi, PL], BF16, tag="eoc", name=f"s7_{b}_{p}")
                    dc = dcp.tile([128, bi, 2, 264], BF16, tag="dc", name=f"dc_{b}_{p}")
                    eo_cascade(
                        tbv[:, :, 0], tbv[:, :, 1], p2[:, :], s2[:, :], s4[:, :],
                        s7[:, :], dc[:, :, 0], dc[:, :, 1],
                    )
                    sum_matmuls(dc[:, :, :, 0:263], [(0, 128), (128, 256), (256, 263)])

            sacc = psp.tile([1, 512], F32, tag="sacc")

            # ---- interleaved emission: no engine stream ever waits on a
            # later-emitted op of another engine ----
            pass1(0)
            evac(0)
            pass1(1)
            pass2(0)
            evac(1)
            pass2(1)

            # ---- packed last W-tile: one eo cascade for all images ----
            gp2 = cascp.tile([128, 1, PL], BF16, tag="gcasc", name="gp2")
            gs2 = cascp.tile([128, 1, PL], BF16, tag="gcasc", name="gs2")
            gs4 = cascp.tile([128, 1, PL], BF16, tag="gcasc", name="gs4")
            gs7 = cascp.tile([128, 1, PL], BF16, tag="gcasc", name="gs7")
            gdc = dcp.tile([128, 1, 2, 264], BF16, tag="gdc", name="gdc")
            eo_cascade(
                tbP4[:, :, 0], tbP4[:, :, 1], gp2[:, :], gs2[:, :], gs4[:, :],
                gs7[:, :], gdc[:, :, 0], gdc[:, :, 1],
            )
            sum_matmuls(gdc[:, :, :, 0:263], [(0, 256), (256, 263)], last=True)

            res = accp.tile([1, 512], F32, tag="res")
            nc.scalar.copy(res[:, :], sacc[:, :])
            nc.sync.dma_start(out[:, :], res[:, :])

    return nc


_PROGRAM = None


def _get_program():
    global _PROGRAM
    if _PROGRAM is None:
        _PROGRAM = build_program()
        _PROGRAM.finalize()  # run Bacc passes (wait splitting, regalloc)
    return _PROGRAM


def kernel(generated_image):
    x = np.ascontiguousarray(np.asarray(generated_image), dtype=np.float32)
    assert x.shape == (B, C, H, W)
    nc = _get_program()
    shards = x.reshape(N_CORES, N_IMG, C, H, W)
    in_maps = [{"x": np.ascontiguousarray(shards[i])} for i in range(N_CORES)]
    res = run_bass_kernel_spmd(nc, in_maps, list(range(N_CORES)))
    total = float(np.sum([r["out"].astype(np.float64).sum() for r in res.results]))
    return np.array(-total / DEN, dtype=np.float32)


# revision 22
# speedup vs baseline: 1.0676x; 1.0676x over previous
"""Trainium2 Bass kernel: DarkChannelLoss.

Computes -mean(dark_channel(x)) for x [32,3,512,512] f32, where
dark_channel = reflect-pad(7) -> min over channels -> 15x15 sliding-window
min (windows clipped at bottom/right, i.e. +inf padded by 14).

Sharding: pure data parallel over batch, 4 images per NeuronCore x 8 cores.
Each core computes column-partial sums of its dark-channel map (f32, via
TensorE ones-matmul accumulation in PSUM); the host combines them into the
final scalar mean.

Per-core pipeline (shapes hardcoded), images processed in pairs (bi=2):
  load:   8 up-front SWDGE cast DMAs (f32->bf16), one per (pair, row-tile):
          1.5 MB each, all resident in SBUF (48 KB/partition). Issued before
          any compute so GpSimd descriptor-gen never contends with DVE
          2-port instructions mid-kernel.
  pass 1: per row-tile: channel-min, reflect pads along W (ACT reversed
          copies), sliding-min cascade along W (windows 2,4,8,15), ops
          carry both images of the pair via a middle AP dim.
  transpose: TensorE 128x128 blocks (identity matmul) into per-W-tile PSUM
          banks [128, (img,rowtile), 128]; ScalarE evacuation per W-tile.
  pass 2: per W-tile: sliding-min cascade along H; dc tiles are summed by
          TensorE matmul with a ones-vector into one PSUM f32 accumulator
          (start/stop chain) -- zero DVE cost. The narrow last W-tile
          (14 cols) of all 4 images is packed into one tile at partition
          offsets {0,32,64,96} via ScalarE cross-quad copies, swept once.
  out:    [1, 512] f32 partial column sums; host reduces.

Emission order interleaves ACT evacuations between the two pairs' DVE
pass-1 blocks so no engine stream ever stalls on a later-emitted op.
"""

import numpy as np

try:
    import concourse.bass as bass
except ImportError:  # pragma: no cover
    import sys

    sys.path.insert(0, "/opt/trn_rl_repo")
    import concourse.bass as bass

import concourse.mybir as mybir
import concourse.bacc as bacc
from concourse.tile import TileContext
from concourse.bass_utils import run_bass_kernel_spmd

F32 = mybir.dt.float32
BF16 = mybir.dt.bfloat16
INF = float("inf")
MIN = mybir.AluOpType.min

B, C, H, W = 32, 3, 512, 512
WIN = 15
PAD = WIN // 2          # 7
HP = H + 2 * PAD        # 526 padded rows
WP = W + 2 * PAD        # 526 padded cols
N_CORES = 8
N_IMG = B // N_CORES    # 4 images per core
NT = H // 128           # 4 row tiles of source rows
PT = (WP + 127) // 128  # 5 W tiles
FREE = 544              # rm tile free width
MF = 544                # m/cascade tile free width (col = src_w + 8)
PL = 272                # even/odd half-plane length (270 used: rows 0..539)
DEN = B * HP * WP


def build_program(n_img=N_IMG, bi=2):
    assert n_img % bi == 0
    nb = n_img // bi
    nc = bacc.Bacc("TRN2", target_bir_lowering=False, debug=False)
    x = nc.dram_tensor("x", [n_img, C, H, W], F32, kind="ExternalInput")
    out = nc.dram_tensor("out", [1, 512], F32, kind="ExternalOutput")

    n2w = WP + WIN - 2  # 539
    n4w = n2w - 2
    n8w = n4w - 4

    with TileContext(nc) as tc:
        from contextlib import ExitStack

        with ExitStack() as ctx:
            constp = ctx.enter_context(tc.tile_pool(name="const", bufs=1))
            chp = ctx.enter_context(tc.tile_pool(name="ch", bufs=2))
            tmpp = ctx.enter_context(tc.tile_pool(name="tmp", bufs=2))
            mp = ctx.enter_context(tc.tile_pool(name="m", bufs=2))
            cascp = ctx.enter_context(tc.tile_pool(name="casc", bufs=4))
            rmp = ctx.enter_context(tc.tile_pool(name="rm", bufs=3))
            tbp = ctx.enter_context(tc.tile_pool(name="tb", bufs=1))
            dcp = ctx.enter_context(tc.tile_pool(name="dc", bufs=3))
            accp = ctx.enter_context(tc.tile_pool(name="acc", bufs=1))
            psp = ctx.enter_context(tc.tile_pool(name="ps", bufs=1, space="PSUM"))

            # ---- all loads up-front: one SWDGE cast DMA per (pair, row-tile),
            # emitted FIRST so SWDGE descriptor-gen starts at program start.
            # tag per row-tile, bufs=2 -> each (pair, row-tile) gets its own
            # resident buffer (48 KB/partition total)
            ch = [
                [
                    chp.tile([128, bi, C, W], BF16, tag=f"ch{t}", name=f"ch_{b}_{t}")
                    for t in range(NT)
                ]
                for b in range(nb)
            ]
            # iota first: ident feeds every transpose; it must not queue
            # behind the load descriptor generation on GpSimd
            idt = constp.tile([128, 128], mybir.dt.int16, tag="idt")
            nc.gpsimd.iota(idt[:, :], pattern=[[1, 128]], base=0, channel_multiplier=-1)

            # Hold descriptor generation of chunks >= 2 until chunk 0 has
            # landed: with many chunks queued the 16 SDMA engines round-robin
            # across them at packet granularity, smearing chunk 1's completion
            # past the point pass 1 needs it. This gives chunk 1 exclusive
            # wire time; later chunks have slack.
            pace = constp.tile([128, 1], BF16, tag="pace")
            chunks = [(b, t) for b in range(nb) for t in range(NT)]
            for k, (b, t) in enumerate(chunks):
                if k == 2:
                    nc.gpsimd.tensor_scalar_add(
                        pace[0:1, 0:1], ch[0][0][0:1, 0, 0, 0:1], 0.0
                    )
                nc.gpsimd.dma_start(
                    ch[b][t][:, :, :, :],
                    x[bi * b : bi * (b + 1), :, 128 * t : 128 * (t + 1), :]
                    .rearrange("b c p w -> p b c w"),
                )

            ident = constp.tile([128, 128], BF16, tag="ident")
            nc.vector.tensor_single_scalar(
                ident[:, :], idt[:, :], 0, mybir.AluOpType.is_equal
            )
            ones = constp.tile([128, 1], BF16, tag="ones")
            nc.vector.memset(ones[:, :], 1.0)

            # Transposed buffers hold H rows split into even/odd half-planes
            # (plane length PL): row r=2j   -> plane 0 index j
            #                    row r=2j+1 -> plane 1 index j
            # This keeps every pass-2 operand unit-stride for DVE 2x mode.
            # packed tile for the narrow last W-tile of all n_img images:
            # image i sits at partitions 32i..32i+14; other lanes stay 0.0
            tbP4 = accp.tile([128, 1, 2, PL], BF16, tag="tbP4")
            nc.vector.memset(tbP4[:, :, :, :], 0.0)

            # persistent transposed buffers, one per pair; INF tail for the
            # clipped bottom windows set once here (rows 526..539 = planes
            # j in [263, 270))
            tbm = [
                tbp.tile([128, PT - 1, bi, 2, PL], BF16, tag=f"tbm{b}", name=f"tbm{b}")
                for b in range(nb)
            ]
            tb4 = [
                tbp.tile([128, bi, 2, PL], BF16, tag=f"tb4{b}", name=f"tb4{b}")
                for b in range(nb)
            ]
            for b in range(nb):
                nc.vector.memset(tbm[b][:, :, :, :, 263:PL], INF)
                nc.vector.memset(tb4[b][0:14, :, :, 263:PL], INF)

            pst = [
                [
                    psp.tile(
                        [128, bi, NT, 128], BF16, tag=f"pst{p}", name=f"pst_{b}_{p}"
                    )
                    for p in range(PT)
                ]
                for b in range(nb)
            ]

            def pass1(b):
                for t in range(NT):
                    tmp = tmpp.tile([128, bi, W], BF16, tag="tmp", name=f"tmp_{b}_{t}")
                    nc.vector.tensor_tensor(
                        tmp[:, :, :], ch[b][t][:, :, 0], ch[b][t][:, :, 1], MIN
                    )
                    m = mp.tile([128, bi, MF], BF16, tag="m", name=f"m_{b}_{t}")
                    nc.vector.memset(m[:, :, 8 + W : MF], INF)
                    nc.vector.tensor_tensor(
                        m[:, :, 8 : 8 + W], tmp[:, :, :], ch[b][t][:, :, 2], MIN
                    )
                    # reflect pads: padded 0..6 <- cols 15..9; 519..525 <- 518..512
                    # done on DVE (min(x,x)=x) so the ACT stream has no
                    # pass-1 dependencies and evacuations never stall it
                    nc.vector.tensor_tensor(
                        m[:, :, 1:8], m[:, :, 15:8:-1], m[:, :, 15:8:-1], MIN
                    )
                    nc.vector.tensor_tensor(
                        m[:, :, 520:527], m[:, :, 518:511:-1], m[:, :, 518:511:-1], MIN
                    )

                    w2 = cascp.tile([128, bi, MF], BF16, tag="casc", name=f"w2_{b}_{t}")
                    w4 = cascp.tile([128, bi, MF], BF16, tag="casc", name=f"w4_{b}_{t}")
                    w8 = cascp.tile([128, bi, MF], BF16, tag="casc", name=f"w8_{b}_{t}")
                    nc.vector.tensor_tensor(
                        w2[:, :, 0:n2w], m[:, :, 1 : n2w + 1], m[:, :, 2 : n2w + 2], MIN
                    )
                    nc.vector.tensor_tensor(
                        w4[:, :, 0:n4w], w2[:, :, 0:n4w], w2[:, :, 2 : n4w + 2], MIN
                    )
                    nc.vector.tensor_tensor(
                        w8[:, :, 0:n8w], w4[:, :, 0:n8w], w4[:, :, 4 : n8w + 4], MIN
                    )
                    rm = rmp.tile([128, bi, FREE], BF16, tag="rm", name=f"rm_{b}_{t}")
                    nc.vector.tensor_tensor(
                        rm[:, :, 0:512], w8[:, :, 0:512], w8[:, :, PAD : 512 + PAD], MIN
                    )
                    nc.vector.tensor_tensor(
                        rm[:, :, 512:WP], w8[:, :, 512:WP], w8[:, :, 512 + PAD : WP + PAD], MIN
                    )
                    # transposes for this row-tile: 2 images x (4 full 128-col
                    # W-blocks + one narrow 14-col block)
                    for ii in range(bi):
                        for p in range(PT - 1):
                            nc.tensor.transpose(
                                pst[b][p][:, ii, t, :],
                                rm[:, ii, 128 * p : 128 * (p + 1)],
                                ident[:, :],
                            )
                        nc.tensor.transpose(
                            pst[b][PT - 1][0:14, ii, t, :],
                            rm[:, ii, 512:WP],
                            ident[:, :],
                        )

            def evac(b):
                # PSUM rows r = 128t + w + PAD:
                #   w odd  (w=2u+1) -> even plane, j = 64t + u + 4
                #   w even (w=2u)   -> odd plane,  j = 64t + u + 3
                for p in range(PT):
                    if p < PT - 1:
                        src, npart = pst[b][p], 128
                        tbv = tbm[b][:, p]
                    else:
                        src, npart = pst[b][p][0:14], 14
                        tbv = tb4[b][0:14, :]
                    nc.scalar.copy(
                        tbv[:, :, 0, 4:260].rearrange("a b (t u) -> a b t u", t=NT),
                        src[0:npart, :, :, 1:128:2],
                    )
                    nc.scalar.copy(
                        tbv[:, :, 1, 3:259].rearrange("a b (t u) -> a b t u", t=NT),
                        src[0:npart, :, :, 0:128:2],
                    )
                    # row reflection per plane:
                    #   top: rows 0..6 <- 14..8; bottom: rows 519..525 <- 517..511
                    nc.scalar.copy(tbv[:, :, 0, 0:4], tbv[:, :, 0, 7:3:-1])
                    nc.scalar.copy(tbv[:, :, 1, 0:3], tbv[:, :, 1, 6:3:-1])
                    nc.scalar.copy(tbv[:, :, 0, 260:263], tbv[:, :, 0, 258:255:-1])
                    nc.scalar.copy(tbv[:, :, 1, 259:263], tbv[:, :, 1, 258:254:-1])
                    if p == PT - 1:
                        for ii in range(bi):
                            i = bi * b + ii
                            nc.scalar.copy(
                                tbP4[32 * i : 32 * i + 14, 0, :, :],
                                tb4[b][0:14, ii, :, :],
                            )

            mm_state = {"first": True}

            def sum_matmuls(dc_ap, chunks, last=False):
                # accumulate column sums of dc_ap chunks (slices of the last
                # dim) into sacc via ones-matmul; chunk free size must be <=512
                outer = 1
                for d in dc_ap.shape[1:-1]:
                    outer *= d
                for ci, (lo, hi) in enumerate(chunks):
                    n = outer * (hi - lo)
                    rhs = dc_ap[..., lo:hi]
                    nc.tensor.matmul(
                        sacc[0:1, 0:n],
                        ones[:, :],
                        rhs,
                        start=mm_state["first"],
                        stop=last and ci == len(chunks) - 1,
                        skip_group_check=True,
                    )
                    mm_state["first"] = False

            def eo_cascade(E, O, p2, s2, s4, s7, dce, dco):
                # sliding-min-15 over rows via even/odd planes ([128, x, PL] APs):
                #   p2[j]=min(rows 2j,2j+1); s7[j]=min(p2[j..j+6])
                #   dc_e[j]=min(s7[j],   E[j+7])  (rows 2j..2j+14)
                #   dc_o[j]=min(O[j],    s7[j+1]) (rows 2j+1..2j+15)
                nc.vector.tensor_tensor(p2[:, :, 0:270], E[:, :, 0:270], O[:, :, 0:270], MIN)
                nc.vector.tensor_tensor(s2[:, :, 0:269], p2[:, :, 0:269], p2[:, :, 1:270], MIN)
                nc.vector.tensor_tensor(s4[:, :, 0:267], s2[:, :, 0:267], s2[:, :, 2:269], MIN)
                nc.vector.tensor_tensor(s7[:, :, 0:264], s4[:, :, 0:264], s4[:, :, 3:267], MIN)
                nc.vector.tensor_tensor(dce[:, :, 0:263], s7[:, :, 0:263], E[:, :, 7:270], MIN)
                nc.vector.tensor_tensor(dco[:, :, 0:263], O[:, :, 0:263], s7[:, :, 1:264], MIN)

            def pass2(b):
                for p in range(PT - 1):
                    tbv = tbm[b][:, p]
                    p2 = cascp.tile([128, bi, PL], BF16, tag="eoc", name=f"p2_{b}_{p}")
                    s2 = cascp.tile([128, bi, PL], BF16, tag="eoc", name=f"s2_{b}_{p}")
                    s4 = cascp.tile([128, bi, PL], BF16, tag="eoc", name=f"s4_{b}_{p}")
                    s7 = cascp.tile([128, bi, PL], BF16, tag="eoc", name=f"s7_{b}_{p}")
                    dc = dcp.tile([128, bi, 2, 264], BF16, tag="dc", name=f"dc_{b}_{p}")
                    eo_cascade(
                        tbv[:, :, 0], tbv[:, :, 1], p2[:, :], s2[:, :], s4[:, :],
                        s7[:, :], dc[:, :, 0], dc[:, :, 1],
                    )
                    sum_matmuls(dc[:, :, :, 0:263], [(0, 128), (128, 256), (256, 263)])

            sacc = psp.tile([1, 512], F32, tag="sacc")

            # ---- interleaved emission: no engine stream ever waits on a
            # later-emitted op of another engine ----
            pass1(0)
            evac(0)
            pass1(1)
            pass2(0)
            evac(1)
            pass2(1)

            # ---- packed last W-tile: one eo cascade for all images ----
            gp2 = cascp.tile([128, 1, PL], BF16, tag="gcasc", name="gp2")
            gs2 = cascp.tile([128, 1, PL], BF16, tag="gcasc", name="gs2")
            gs4 = cascp.tile([128, 1, PL], BF16, tag="gcasc", name="gs4")
            gs7 = cascp.tile([128, 1, PL], BF16, tag="gcasc", name="gs7")
            gdc = dcp.tile([128, 1, 2, 264], BF16, tag="gdc", name="gdc")
            eo_cascade(
                tbP4[:, :, 0], tbP4[:, :, 1], gp2[:, :], gs2[:, :], gs4[:, :],
                gs7[:, :], gdc[:, :, 0], gdc[:, :, 1],
            )
            sum_matmuls(gdc[:, :, :, 0:263], [(0, 256), (256, 263)], last=True)

            res = accp.tile([1, 512], F32, tag="res")
            nc.scalar.copy(res[:, :], sacc[:, :])
            nc.sync.dma_start(out[:, :], res[:, :])

    return nc


_PROGRAM = None


def _get_program():
    global _PROGRAM
    if _PROGRAM is None:
        _PROGRAM = build_program()
        _PROGRAM.finalize()  # run Bacc passes (wait splitting, regalloc)
    return _PROGRAM


def kernel(generated_image):
    x = np.ascontiguousarray(np.asarray(generated_image), dtype=np.float32)
    assert x.shape == (B, C, H, W)
    nc = _get_program()
    shards = x.reshape(N_CORES, N_IMG, C, H, W)
    in_maps = [{"x": np.ascontiguousarray(shards[i])} for i in range(N_CORES)]
    res = run_bass_kernel_spmd(nc, in_maps, list(range(N_CORES)))
    total = float(np.sum([r["out"].astype(np.float64).sum() for r in res.results]))
    return np.array(-total / DEN, dtype=np.float32)
